# revision 1
# baseline (speedup 1.0000x reference)
"""BiMamba (bidirectional Mamba-1) Trainium2 kernel.

Full inputs -> full outputs. Sharding: 8 cores = (batch 2) x (direction 2) x
(channel-half 2); every core runs the SAME NEFF on different inputs (the
host reverses the sequence for the backward direction, permutes channel
halves, and pre-transposes weights/activations to channel-major).

Per-core pipeline (channel-major [channel, time], chunked over time):
  PE:   in_proj matmuls, depthwise causal conv as diagonal matmuls,
        x_proj, dt_proj, B/C one-hot partition broadcasts, out_proj
  ACT:  softplus = Ln(1+Exp(.)), SiLU, dA = Exp(A * delta) (per-part scale)
  DVE:  dBu = dx*B, hardware linear scan (h = dA*h + dBu), g = h*C,
        carry columns between chunks
  GPSIMD: sum over the 16 states, skip connection, z-gate products
"""

import functools

import ml_dtypes
import numpy as np

import concourse.bass as bass
import concourse.mybir as mybir
import concourse.tile as tile
from concourse.bass_utils import run_bass_kernel_spmd

L = 8192          # sequence length
DM = 512          # d_model
EF = 1024         # d_inner (full)
E2 = 512          # d_inner per core (channel-half)
NST = 16          # d_state
R = 32            # dt_rank
T = 512           # time chunk
NCH = L // T
NB_E2 = E2 // 128     # 4 e-tiles for own half
NB_EF = EF // 128     # 8 e-tiles full
F32 = mybir.dt.float32
BF16 = mybir.dt.bfloat16
AF = mybir.ActivationFunctionType
OP = mybir.AluOpType
TRACE = False      # set by test harness to capture an NTFF profile


def _split_dma_waits(nc):
    """walrus codegen embeds a limited number of sem-waits per instruction
    (1 for dynamic DMA descriptors, 2 for compute engine instructions);
    move extra waits onto NoOps executed by the same engine just before."""
    SKIP = {"EventSemaphore", "UnconditionalBranch", "Call",
            "RegisterMove", "NoOp", "ISA"}
    f0 = nc.m.functions[0]
    plan = {}
    for blk in f0.blocks:
        for inst in list(blk.instructions):
            if inst.opcode in SKIP or inst.sync_info is None:
                continue
            cap = 1
            if len(inst.sync_info.on_wait) > cap:
                plan[inst.name] = (inst, cap)
    if not plan:
        return
    existing = {i.name for blk in f0.blocks for i in blk.instructions}

    def new_nop(engine, waits):
        nc.engines[engine].nop()
        newn = None
        for blk in f0.blocks:
            for i in blk.instructions:
                if i.name not in existing:
                    newn = i
                    existing.add(i.name)
        assert newn is not None
        newn.sync_info = mybir.SyncInfo(on_wait=waits, on_update=[])
        return newn

    nops_for = {}
    for name, (inst, cap) in plan.items():
        si = inst.sync_info
        waits = list(si.on_wait)
        if inst.opcode == "DMACopy":
            keep = [w for w in waits if "DMA" in (w.ant_name or "")][-cap:]
            if not keep:
                keep = waits[-cap:]
        else:
            keep = waits[-cap:]
        moved = [w for w in waits if not any(w is k for k in keep)]
        nops = []
        for i in range(0, len(moved), 1):
            nops.append(new_nop(inst.engine, moved[i:i + 1]))
        inst.sync_info = mybir.SyncInfo(on_wait=keep,
                                        on_update=list(si.on_update))
        nops_for[name] = nops
    nop_names = {n.name for ns in nops_for.values() for n in ns}
    for blk in f0.blocks:
        lst = []
        for inst in blk.instructions:
            if inst.name in nop_names:
                continue
            if inst.name in plan:
                lst.extend(nops_for[inst.name])
            lst.append(inst)
        blk.instructions = lst


def build_core_program(L=L, T=T, num_devices=8):
    NCH = L // T
    nc = bass.Bass("TRN2", target_bir_lowering=False, debug=False,
                   num_devices=num_devices)
    # ---- DRAM I/O (per core) ----
    u = nc.dram_tensor("u", [DM, L], BF16, kind="ExternalInput").ap()
    w_in = nc.dram_tensor("w_in", [DM, EF + E2], BF16,
                          kind="ExternalInput").ap()
    dg = nc.dram_tensor("dg", [4, NB_EF, 128, 128], BF16,
                        kind="ExternalInput").ap()
    conv_b = nc.dram_tensor("conv_b", [EF, 1], F32, kind="ExternalInput").ap()
    w_xp = nc.dram_tensor("w_xp", [EF, 96], BF16, kind="ExternalInput").ap()
    w_dt = nc.dram_tensor("w_dt", [R, E2], F32, kind="ExternalInput").ap()
    dt_b = nc.dram_tensor("dt_b", [E2, 1], F32, kind="ExternalInput").ap()
    A_in = nc.dram_tensor("A", [E2, NST], F32, kind="ExternalInput").ap()
    Dp = nc.dram_tensor("Dp", [E2, 1], F32, kind="ExternalInput").ap()
    w_out = nc.dram_tensor("w_out", [E2, DM], F32, kind="ExternalInput").ap()
    oh = nc.dram_tensor("oh", [96, NST * 128], F32,
                        kind="ExternalInput").ap()
    out = nc.dram_tensor("out", [DM, L], F32, kind="ExternalOutput").ap()

    with tile.TileContext(nc) as tc:
        with (
            tc.tile_pool(name="wpool", bufs=1) as wp,
            tc.tile_pool(name="persist", bufs=1) as pp,
            tc.tile_pool(name="stream", bufs=2) as sp,
            tc.tile_pool(name="xother", bufs=1) as xo,
            tc.tile_pool(name="scanp", bufs=3) as scp,
            tc.tile_pool(name="gpool", bufs=2) as gp,
            tc.tile_pool(name="ps_in", bufs=2, space="PSUM") as ps_in,
            tc.tile_pool(name="ps_conv", bufs=2, space="PSUM") as ps_conv,
            tc.tile_pool(name="ps_small", bufs=2, space="PSUM") as ps_small,
            tc.tile_pool(name="ps_bc", bufs=1, space="PSUM") as ps_bc,
        ):
            # ---------------- weights into SBUF ----------------
            w_in_sb = [wp.tile([128, EF + E2], BF16, tag=f"w_in{k}", name=f"w_in{k}")
                       for k in range(4)]
            for k in range(4):
                nc.sync.dma_start(w_in_sb[k], w_in[k * 128:(k + 1) * 128, :])
            dg_sb = [[wp.tile([128, 128], BF16, tag=f"dg{k}_{eb}", name=f"dg{k}_{eb}")
                      for eb in range(NB_EF)] for k in range(4)]
            for k in range(4):
                for eb in range(NB_EF):
                    nc.sync.dma_start(dg_sb[k][eb], dg[k, eb])
            conv_b_sb = [wp.tile([128, 1], F32, tag=f"cb{eb}", name=f"cb{eb}")
                         for eb in range(NB_EF)]
            for eb in range(NB_EF):
                nc.sync.dma_start(conv_b_sb[eb],
                                  conv_b[eb * 128:(eb + 1) * 128, :])
            w_xp_sb = [wp.tile([128, 96], BF16, tag=f"wxp{eb}", name=f"wxp{eb}")
                       for eb in range(NB_EF)]
            for eb in range(NB_EF):
                nc.sync.dma_start(w_xp_sb[eb], w_xp[eb * 128:(eb + 1) * 128])
            w_dt_sb = wp.tile([R, E2], F32)
            nc.sync.dma_start(w_dt_sb, w_dt)
            dt_b_sb = [wp.tile([128, 1], F32, tag=f"dtb{et}", name=f"dtb{et}")
                       for et in range(NB_E2)]
            A_sb = [wp.tile([128, NST], F32, tag=f"A{et}", name=f"A{et}")
                    for et in range(NB_E2)]
            Dp_sb = [wp.tile([128, 1], F32, tag=f"Dp{et}", name=f"Dp{et}")
                     for et in range(NB_E2)]
            for et in range(NB_E2):
                nc.sync.dma_start(dt_b_sb[et], dt_b[et * 128:(et + 1) * 128])
                nc.sync.dma_start(A_sb[et], A_in[et * 128:(et + 1) * 128])
                nc.sync.dma_start(Dp_sb[et], Dp[et * 128:(et + 1) * 128])
            w_out_sb = [wp.tile([128, DM], F32, tag=f"wo{k}", name=f"wo{k}")
                        for k in range(NB_E2)]
            for k in range(NB_E2):
                nc.sync.dma_start(w_out_sb[k], w_out[k * 128:(k + 1) * 128])
            oh_sb = wp.tile([96, NST * 128], F32)
            nc.sync.dma_start(oh_sb, oh)

            # persistent state
            xpre_sb = [pp.tile([128, T + 3], BF16, tag=f"xpre{eb}", name=f"xpre{eb}")
                       for eb in range(NB_EF)]
            carry = [pp.tile([128, NST], F32, tag=f"carry{et}", name=f"carry{et}")
                     for et in range(NB_E2)]
            for eb in range(NB_EF):
                nc.vector.memset(xpre_sb[eb][:, 0:3], 0.0)

            # ---------------- chunk loop ----------------
            for c in range(NCH):
                t0 = c * T
                # halo: carry last 3 cols of previous chunk
                if c > 0:
                    for eb in range(NB_EF):
                        nc.vector.tensor_copy(
                            out=xpre_sb[eb][:, 0:3],
                            in_=xpre_sb[eb][:, T:T + 3])
                # u chunk in
                u_sb = [sp.tile([128, T], BF16, tag=f"u{k}", name=f"u{k}")
                        for k in range(4)]
                for k in range(4):
                    nc.sync.dma_start(
                        u_sb[k], u[k * 128:(k + 1) * 128, t0:t0 + T])
                # in_proj: 12 m-blocks (8 x-part, 4 z-own)
                sz = []
                for m in range(12):
                    acc = ps_in.tile([128, T], F32, tag="inproj")
                    for k in range(4):
                        nc.tensor.matmul(
                            acc, w_in_sb[k][:, m * 128:(m + 1) * 128],
                            u_sb[k], start=(k == 0), stop=(k == 3))
                    if m < 8:
                        nc.scalar.copy(out=xpre_sb[m][:, 3:T + 3], in_=acc)
                    else:
                        szt = sp.tile([128, T], BF16, tag=f"sz{m - 8}")
                        nc.scalar.activation(out=szt, in_=acc, func=AF.Silu)
                        sz.append(szt)
                # conv (diag matmuls) + bias + silu -> xT
                xT = []
                for eb in range(NB_EF):
                    accc = ps_conv.tile([128, T], F32, tag="conv")
                    for k in range(4):
                        nc.tensor.matmul(
                            accc, dg_sb[k][eb], xpre_sb[eb][:, k:k + T],
                            start=(k == 0), stop=(k == 3))
                    pool = sp if eb < NB_E2 else xo
                    xt = pool.tile([128, T], BF16, tag=f"xT{eb}")
                    nc.scalar.activation(out=xt, in_=accc, func=AF.Silu,
                                         bias=conv_b_sb[eb][:, 0:1])
                    xT.append(xt)
                # x_proj -> xdbl [96, T]
                xdbl_ps = ps_small.tile([96, T], F32, tag="small")
                for eb in range(NB_EF):
                    nc.tensor.matmul(xdbl_ps, w_xp_sb[eb], xT[eb],
                                     start=(eb == 0), stop=(eb == 7))
                xdbl = sp.tile([96, T], F32)
                nc.scalar.copy(out=xdbl, in_=xdbl_ps)
                # dt_proj + softplus -> delta (per e-tile)
                delta = []
                dx = []
                for et in range(NB_E2):
                    dpre = ps_small.tile([128, T], F32, tag="small")
                    nc.tensor.matmul(
                        dpre, w_dt_sb[:, et * 128:(et + 1) * 128],
                        xdbl[0:R, :], start=True, stop=True)
                    dl = sp.tile([128, T], F32, tag=f"delta{et}")
                    # e = exp(pre + dt_b); delta = ln(1 + e)
                    nc.scalar.activation(out=dl, in_=dpre, func=AF.Exp,
                                         bias=dt_b_sb[et][:, 0:1])
                    nc.scalar.activation(out=dl, in_=dl, func=AF.Ln,
                                         bias=1.0)
                    delta.append(dl)
                    dxt = sp.tile([128, T], F32, tag=f"dx{et}")
                    nc.vector.tensor_tensor(out=dxt, in0=dl, in1=xT[et],
                                            op=OP.mult)
                    dx.append(dxt)
                # y accumulators
                y = []
                for et in range(NB_E2):
                    yt = sp.tile([128, T], F32, tag=f"y{et}", name=f"y{et}")
                    nc.gpsimd.memset(yt, 0.0)
                    y.append(yt)
                # scan over states
                for n in range(NST):
                    bcB = ps_bc.tile([128, T], F32, tag="bcB")
                    nc.tensor.matmul(
                        bcB, oh_sb[32:48, n * 128:(n + 1) * 128],
                        xdbl[32:48, :], start=True, stop=True)
                    bcC = ps_bc.tile([128, T], F32, tag="bcC")
                    nc.tensor.matmul(
                        bcC, oh_sb[64:80, n * 128:(n + 1) * 128],
                        xdbl[64:80, :], start=True, stop=True)
                    for et in range(NB_E2):
                        g = gp.tile([128, T], F32, tag=f"g{et}")
                        dA = scp.tile([128, T], F32, tag="dA")
                        nc.scalar.activation(
                            out=dA, in_=delta[et], func=AF.Exp,
                            scale=A_sb[et][:, n:n + 1])
                        dBu = scp.tile([128, T], F32, tag="dBu")
                        nc.vector.tensor_tensor(
                            out=dBu, in0=dx[et], in1=bcB, op=OP.mult)
                        nc.vector.tensor_tensor_scan(
                            out=g, data0=dA, data1=dBu,
                            initial=(0.0 if c == 0
                                     else carry[et][:, n:n + 1]),
                            op0=OP.mult, op1=OP.add)
                        nc.vector.tensor_copy(
                            out=carry[et][:, n:n + 1],
                            in_=g[:, T - 1:T])
                        nc.vector.tensor_tensor(
                            out=g, in0=g, in1=bcC, op=OP.mult)
                        nc.gpsimd.tensor_tensor(
                            out=y[et], in0=y[et], in1=g, op=OP.add)
                # skip + gate: y = (y + x*D) * silu(z)
                for et in range(NB_E2):
                    nc.vector.scalar_tensor_tensor(
                        out=y[et], in0=xT[et], scalar=Dp_sb[et][:, 0:1],
                        in1=y[et], op0=OP.mult, op1=OP.add)
                    nc.vector.tensor_tensor(
                        out=y[et], in0=y[et], in1=sz[et], op=OP.mult)
                # out_proj -> out[:, chunk]
                for m in range(DM // 128):
                    acco = ps_small.tile([128, T], F32, tag="small")
                    for k in range(NB_E2):
                        nc.tensor.matmul(
                            acco, w_out_sb[k][:, m * 128:(m + 1) * 128],
                            y[k], start=(k == 0), stop=(k == 3))
                    osb = sp.tile([128, T], F32, tag="osb")
                    nc.scalar.copy(out=osb, in_=acco)
                    nc.sync.dma_start(
                        out[m * 128:(m + 1) * 128, t0:t0 + T], osb)
    _split_dma_waits(nc)
    return nc


@functools.lru_cache(maxsize=1)
def _get_program():
    return build_core_program()


def _prep_core_inputs(hs, in_w, out_w, conv_w, conv_b, xproj_w, dt_w, dt_b,
                      A_log, D, b, rev, eh):
    bf = ml_dtypes.bfloat16
    own = slice(eh * E2, (eh + 1) * E2)
    other = slice((1 - eh) * E2, (2 - eh) * E2)
    perm = np.r_[np.arange(eh * E2, (eh + 1) * E2),
                 np.arange((1 - eh) * E2, (2 - eh) * E2)]
    u = hs[b] if not rev else hs[b, ::-1]
    u_t = np.ascontiguousarray(u.T).astype(bf)                  # [DM, L]
    w_x = in_w[:EF][perm]                                       # [EF, DM]
    w_z = in_w[EF:][own]                                        # [E2, DM]
    w_in_t = np.ascontiguousarray(
        np.concatenate([w_x, w_z], 0).T).astype(bf)             # [DM, 1536]
    cw = conv_w[:, 0, :][perm]                                  # [EF, 4]
    dg = np.zeros((4, NB_EF, 128, 128), bf)
    for k in range(4):
        for eb in range(NB_EF):
            dg[k, eb] = np.diag(cw[eb * 128:(eb + 1) * 128, k]).astype(bf)
    cb = np.ascontiguousarray(conv_b[perm][:, None]).astype(np.float32)
    xp = xproj_w[:, perm]                                       # [64, EF]
    xp_pad = np.zeros((96, EF), np.float32)
    xp_pad[0:32] = xp[0:32]        # dt
    xp_pad[32:48] = xp[32:48]      # B
    xp_pad[64:80] = xp[48:64]      # C
    w_xp_t = np.ascontiguousarray(xp_pad.T).astype(bf)          # [EF, 96]
    w_dt_t = np.ascontiguousarray(dt_w[own].T).astype(np.float32)  # [R, E2]
    dtb = np.ascontiguousarray(dt_b[own][:, None]).astype(np.float32)
    A = (-np.exp(A_log[own])).astype(np.float32)                # [E2, NST]
    Dpv = np.ascontiguousarray(D[own][:, None]).astype(np.float32)
    w_out_t = np.ascontiguousarray(out_w[:, own].T).astype(np.float32)
    ohm = np.zeros((96, NST * 128), np.float32)
    for n in range(NST):
        ohm[32 + n, n * 128:(n + 1) * 128] = 1.0
        ohm[64 + n, n * 128:(n + 1) * 128] = 1.0
    return dict(u=u_t, w_in=w_in_t, dg=dg, conv_b=cb, w_xp=w_xp_t,
                w_dt=w_dt_t, dt_b=dtb, A=A, Dp=Dpv, w_out=w_out_t, oh=ohm)


def kernel(hidden_states, in_proj_w, out_proj_w,
           conv_w_f, conv_b_f, xproj_w_f, dtproj_w_f, dtproj_b_f, A_log_f,
           D_f, conv_w_r, conv_b_r, xproj_w_r, dtproj_w_r, dtproj_b_r,
           A_log_r, D_r, _results_hook=None):
    hs = np.asarray(hidden_states, np.float32)
    params = {
        False: (conv_w_f, conv_b_f, xproj_w_f, dtproj_w_f, dtproj_b_f,
                A_log_f, D_f),
        True: (conv_w_r, conv_b_r, xproj_w_r, dtproj_w_r, dtproj_b_r,
               A_log_r, D_r),
    }
    cores = []          # (b, rev, eh)
    in_maps = []
    for b in range(2):
        for rev in (False, True):
            for eh in range(2):
                cw, cb, xw, dw, db, al, dd = [np.asarray(p, np.float32)
                                              for p in params[rev]]
                in_maps.append(_prep_core_inputs(
                    hs, np.asarray(in_proj_w, np.float32),
                    np.asarray(out_proj_w, np.float32),
                    cw, cb, xw, dw, db, al, dd, b, rev, eh))
                cores.append((b, rev, eh))
    nc = _get_program()
    res = run_bass_kernel_spmd(nc, in_maps, core_ids=list(range(8)),
                               trace=TRACE)
    if _results_hook is not None:
        _results_hook(res)
    out = np.zeros((2, L, DM), np.float32)
    for (b, rev, eh), r in zip(cores, res.results):
        part = r["out"].T            # [L, DM]
        if rev:
            part = part[::-1]
        out[b] += part
    return out



# revision 6
# speedup vs baseline: 1.1069x; 1.1069x over previous
"""BiMamba (bidirectional Mamba-1) Trainium2 kernel.

Full inputs -> full outputs. Sharding: 8 cores = (batch 2) x (direction 2) x
(channel-half 2); every core runs the SAME NEFF on different inputs (the
host reverses the sequence for the backward direction, permutes channel
halves, and pre-transposes weights/activations to channel-major).

Per-core pipeline (channel-major [channel, time], chunked over time).
Engine assignment tuned from HW traces (DVE is the bottleneck):
  PE:   in_proj, depthwise conv as diagonal matmuls, x_proj, dt_proj,
        B/C one-hot partition broadcasts, y state-sum via identity-matmul
        PSUM accumulation, out_proj  (all bf16)
  ACT:  softplus = Ln(1+Exp(.)), SiLU, dA = Exp(A*delta), PSUM->SBUF
        bf16 copies
  DVE:  dBu = dx*B and g = h*C as bf16 2x-mode TTs over 4-state
        concatenated tiles, the hardware linear scan (fp32 internal
        state) over [128, 4*T] with per-state dA-column resets, carry
        fold/extract as tiny strided ops
"""

import functools

import ml_dtypes
import numpy as np

import concourse.bass as bass
import concourse.mybir as mybir
import concourse.tile as tile
from concourse.tile import add_dep_helper
from concourse.bass_utils import run_bass_kernel_spmd

L = 8192          # sequence length
DM = 512          # d_model
EF = 1024         # d_inner (full)
E2 = 512          # d_inner per core (channel-half)
NST = 16          # d_state
R = 32            # dt_rank
T = 512           # time chunk
NCH = L // T
NG = 4            # states per scan group
NGRP = NST // NG  # groups
NB_E2 = E2 // 128     # 4 e-tiles for own half
NB_EF = EF // 128     # 8 e-tiles full
F32 = mybir.dt.float32
BF16 = mybir.dt.bfloat16
AF = mybir.ActivationFunctionType
OP = mybir.AluOpType
TRACE = False      # set by test harness to capture an NTFF profile


def _split_dma_waits(nc):
    """walrus codegen embeds a limited number of sem-waits per instruction
    (1 for dynamic DMA descriptors, 2 for compute engine instructions);
    move extra waits onto NoOps executed by the same engine just before."""
    SKIP = {"EventSemaphore", "UnconditionalBranch", "Call",
            "RegisterMove", "NoOp", "ISA"}
    f0 = nc.m.functions[0]
    plan = {}
    for blk in f0.blocks:
        for inst in list(blk.instructions):
            if inst.opcode in SKIP or inst.sync_info is None:
                continue
            cap = 1
            if len(inst.sync_info.on_wait) > cap:
                plan[inst.name] = (inst, cap)
    if not plan:
        return
    existing = {i.name for blk in f0.blocks for i in blk.instructions}

    def new_nop(engine, waits):
        nc.engines[engine].nop()
        newn = None
        for blk in f0.blocks:
            for i in blk.instructions:
                if i.name not in existing:
                    newn = i
                    existing.add(i.name)
        assert newn is not None
        newn.sync_info = mybir.SyncInfo(on_wait=waits, on_update=[])
        return newn

    nops_for = {}
    for name, (inst, cap) in plan.items():
        si = inst.sync_info
        waits = list(si.on_wait)
        if inst.opcode == "DMACopy":
            keep = [w for w in waits if "DMA" in (w.ant_name or "")][-cap:]
            if not keep:
                keep = waits[-cap:]
        else:
            keep = waits[-cap:]
        moved = [w for w in waits if not any(w is k for k in keep)]
        nops = []
        for i in range(0, len(moved), 1):
            nops.append(new_nop(inst.engine, moved[i:i + 1]))
        inst.sync_info = mybir.SyncInfo(on_wait=keep,
                                        on_update=list(si.on_update))
        nops_for[name] = nops
    nop_names = {n.name for ns in nops_for.values() for n in ns}
    for blk in f0.blocks:
        lst = []
        for inst in blk.instructions:
            if inst.name in nop_names:
                continue
            if inst.name in plan:
                lst.extend(nops_for[inst.name])
            lst.append(inst)
        blk.instructions = lst


def build_core_program(L=L, T=T, num_devices=8):
    NCH = L // T
    nc = bass.Bass("TRN2", target_bir_lowering=False, debug=False,
                   num_devices=num_devices)
    # ---- DRAM I/O (per core) ----
    u = nc.dram_tensor("u", [DM, L], BF16, kind="ExternalInput").ap()
    w_in = nc.dram_tensor("w_in", [DM, EF + E2], BF16,
                          kind="ExternalInput").ap()
    dg = nc.dram_tensor("dg", [4, NB_EF, 128, 128], BF16,
                        kind="ExternalInput").ap()
    conv_b = nc.dram_tensor("conv_b", [EF, 1], F32, kind="ExternalInput").ap()
    w_xp = nc.dram_tensor("w_xp", [EF, 96], BF16, kind="ExternalInput").ap()
    w_dt = nc.dram_tensor("w_dt", [R, E2], BF16, kind="ExternalInput").ap()
    dt_b = nc.dram_tensor("dt_b", [E2, 1], F32, kind="ExternalInput").ap()
    A_in = nc.dram_tensor("A", [E2, NST], F32, kind="ExternalInput").ap()
    Dp = nc.dram_tensor("Dp", [E2, 1], F32, kind="ExternalInput").ap()
    w_out = nc.dram_tensor("w_out", [E2, DM], BF16, kind="ExternalInput").ap()
    oh = nc.dram_tensor("oh", [96, NST * 128], BF16,
                        kind="ExternalInput").ap()
    idm = nc.dram_tensor("idm", [128, 128], BF16, kind="ExternalInput").ap()
    out = nc.dram_tensor("out", [DM, L], BF16, kind="ExternalOutput").ap()

    with tile.TileContext(nc) as tc:
        with (
            tc.tile_pool(name="wpool", bufs=1) as wp,
            tc.tile_pool(name="persist", bufs=1) as pp,
            tc.tile_pool(name="stream", bufs=2) as sp,
            tc.tile_pool(name="xother", bufs=2) as xo,
            tc.tile_pool(name="scanp", bufs=2) as scp,
            tc.tile_pool(name="bcpool", bufs=2) as bcp,
            tc.tile_pool(name="ps_main", bufs=2, space="PSUM") as ps_main,
            tc.tile_pool(name="ps_bc", bufs=2, space="PSUM") as ps_bc,
            tc.tile_pool(name="ps_y", bufs=1, space="PSUM") as ps_y,
        ):
            # ---------------- weights into SBUF ----------------
            w_in_sb = [wp.tile([128, EF + E2], BF16, tag=f"w_in{k}",
                               name=f"w_in{k}") for k in range(4)]
            for k in range(4):
                nc.sync.dma_start(w_in_sb[k], w_in[k * 128:(k + 1) * 128, :])
            dg_sb = [[wp.tile([128, 128], BF16, tag=f"dg{k}_{eb}",
                              name=f"dg{k}_{eb}")
                      for eb in range(NB_EF)] for k in range(4)]
            for k in range(4):
                for eb in range(NB_EF):
                    nc.sync.dma_start(dg_sb[k][eb], dg[k, eb])
            conv_b_sb = [wp.tile([128, 1], F32, tag=f"cb{eb}", name=f"cb{eb}")
                         for eb in range(NB_EF)]
            for eb in range(NB_EF):
                nc.sync.dma_start(conv_b_sb[eb],
                                  conv_b[eb * 128:(eb + 1) * 128, :])
            w_xp_sb = [wp.tile([128, 96], BF16, tag=f"wxp{eb}", name=f"wxp{eb}")
                       for eb in range(NB_EF)]
            for eb in range(NB_EF):
                nc.sync.dma_start(w_xp_sb[eb], w_xp[eb * 128:(eb + 1) * 128])
            w_dt_sb = wp.tile([R, E2], BF16)
            nc.sync.dma_start(w_dt_sb, w_dt)
            dt_b_sb = [wp.tile([128, 1], F32, tag=f"dtb{et}", name=f"dtb{et}")
                       for et in range(NB_E2)]
            A_sb = [wp.tile([128, NST], F32, tag=f"A{et}", name=f"A{et}")
                    for et in range(NB_E2)]
            Dp_sb = [wp.tile([128, 1], F32, tag=f"Dp{et}", name=f"Dp{et}")
                     for et in range(NB_E2)]
            for et in range(NB_E2):
                nc.sync.dma_start(dt_b_sb[et], dt_b[et * 128:(et + 1) * 128])
                nc.sync.dma_start(A_sb[et], A_in[et * 128:(et + 1) * 128])
                nc.sync.dma_start(Dp_sb[et], Dp[et * 128:(et + 1) * 128])
            w_out_sb = [wp.tile([128, DM], BF16, tag=f"wo{k}", name=f"wo{k}")
                        for k in range(NB_E2)]
            for k in range(NB_E2):
                nc.sync.dma_start(w_out_sb[k], w_out[k * 128:(k + 1) * 128])
            oh_sb = wp.tile([96, NST * 128], BF16)
            nc.sync.dma_start(oh_sb, oh)
            id_sb = wp.tile([128, 128], BF16)
            nc.sync.dma_start(id_sb, idm)

            # persistent state
            xpre_sb = [pp.tile([128, T + 3], BF16, tag=f"xpre{eb}",
                               name=f"xpre{eb}") for eb in range(NB_EF)]
            carry = [pp.tile([128, NST], BF16, tag=f"carry{et}",
                             name=f"carry{et}") for et in range(NB_E2)]
            for eb in range(NB_EF):
                nc.vector.memset(xpre_sb[eb][:, 0:3], 0.0)

            last_exp_of_chunk = None
            # ---------------- chunk loop ----------------
            for c in range(NCH):
                t0 = c * T
                # halo: carry last 3 cols of previous chunk
                if c > 0:
                    for eb in range(NB_EF):
                        nc.vector.tensor_copy(
                            out=xpre_sb[eb][:, 0:3],
                            in_=xpre_sb[eb][:, T:T + 3])
                # u chunk in
                u_sb = [sp.tile([128, T], BF16, tag=f"u{k}", name=f"u{k}")
                        for k in range(4)]
                for k in range(4):
                    nc.sync.dma_start(
                        u_sb[k], u[k * 128:(k + 1) * 128, t0:t0 + T])
                # in_proj: 12 m-blocks (8 x-part, 4 z-own)
                sz = []
                silu_insts = []
                for m in range(12):
                    acc = ps_main.tile([128, T], F32, tag="mainps")
                    for k in range(4):
                        nc.tensor.matmul(
                            acc, w_in_sb[k][:, m * 128:(m + 1) * 128],
                            u_sb[k], start=(k == 0), stop=(k == 3))
                    if m < 8:
                        nc.scalar.copy(out=xpre_sb[m][:, 3:T + 3], in_=acc)
                    else:
                        szt = sp.tile([128, T], BF16, tag=f"sz{m - 8}")
                        si = nc.scalar.activation(out=szt, in_=acc,
                                                  func=AF.Silu)
                        silu_insts.append(si)
                        sz.append(szt)
                # conv (diag matmuls) + bias + silu -> xT
                xT = []
                for eb in range(NB_EF):
                    accc = ps_main.tile([128, T], F32, tag="mainps")
                    for k in range(4):
                        nc.tensor.matmul(
                            accc, dg_sb[k][eb], xpre_sb[eb][:, k:k + T],
                            start=(k == 0), stop=(k == 3))
                    pool = sp if eb < NB_E2 else xo
                    xt = pool.tile([128, T], BF16, tag=f"xT{eb}")
                    si = nc.scalar.activation(out=xt, in_=accc, func=AF.Silu,
                                              bias=conv_b_sb[eb][:, 0:1])
                    silu_insts.append(si)
                    xT.append(xt)
                # keep ACT table sets clustered: all Silus of this chunk
                # after the previous chunk's last Exp
                if last_exp_of_chunk is not None:
                    for si in silu_insts:
                        add_dep_helper(si.ins, last_exp_of_chunk.ins,
                                       sync=False,
                                       reason="act table-set clustering")
                # x_proj -> xdbl [96, T] -> bf16
                xdbl_ps = ps_main.tile([96, T], F32, tag="mainps")
                for eb in range(NB_EF):
                    nc.tensor.matmul(xdbl_ps, w_xp_sb[eb], xT[eb],
                                     start=(eb == 0), stop=(eb == 7))
                xdbl = sp.tile([96, T], BF16, tag="xdbl")
                nc.scalar.copy(out=xdbl, in_=xdbl_ps)
                # dt_proj + softplus -> delta; dx = delta*x (per e-tile)
                delta = []
                dx = []
                for et in range(NB_E2):
                    dpre = ps_main.tile([128, T], F32, tag="mainps")
                    nc.tensor.matmul(
                        dpre, w_dt_sb[:, et * 128:(et + 1) * 128],
                        xdbl[0:R, :], start=True, stop=True)
                    dl = sp.tile([128, T], BF16, tag=f"delta{et}")
                    # e = exp(pre + dt_b); delta = ln(1 + e)
                    nc.scalar.activation(out=dl, in_=dpre, func=AF.Exp,
                                         bias=dt_b_sb[et][:, 0:1])
                    nc.scalar.activation(out=dl, in_=dl, func=AF.Ln,
                                         bias=1.0)
                    delta.append(dl)
                    dxt = sp.tile([128, T], BF16, tag=f"dx{et}")
                    nc.vector.tensor_tensor(out=dxt, in0=dl, in1=xT[et],
                                            op=OP.mult)
                    dx.append(dxt)
                # scan groups
                y_ps = [ps_y.tile([128, T], F32, tag=f"yps{et}",
                                  name=f"yps{et}")
                        for et in range(NB_E2)]
                for grp in range(NGRP):
                    n0 = grp * NG
                    # B/C broadcasts for the NG states of this group
                    bcB = bcp.tile([128, NG * T], BF16, tag="bcB")
                    bcC = bcp.tile([128, NG * T], BF16, tag="bcC")
                    for j in range(NG):
                        n = n0 + j
                        pb = ps_bc.tile([128, T], F32, tag="bcps")
                        nc.tensor.matmul(
                            pb, oh_sb[32:48, n * 128:(n + 1) * 128],
                            xdbl[32:48, :], start=True, stop=True)
                        nc.scalar.copy(out=bcB[:, j * T:(j + 1) * T], in_=pb)
                        pc2 = ps_bc.tile([128, T], F32, tag="bcps")
                        nc.tensor.matmul(
                            pc2, oh_sb[64:80, n * 128:(n + 1) * 128],
                            xdbl[64:80, :], start=True, stop=True)
                        nc.scalar.copy(out=bcC[:, j * T:(j + 1) * T], in_=pc2)
                    for et in range(NB_E2):
                        dA = scp.tile([128, NG * T], BF16, tag="dA")
                        dBu = scp.tile([128, NG * T], BF16, tag="dBu")
                        h = scp.tile([128, NG * T], BF16, tag="h")
                        g = scp.tile([128, NG * T], BF16, tag="g")
                        for j in range(NG):
                            n = n0 + j
                            ei = nc.scalar.activation(
                                out=dA[:, j * T:(j + 1) * T],
                                in_=delta[et], func=AF.Exp,
                                scale=A_sb[et][:, n:n + 1])
                            last_exp_of_chunk = ei
                            nc.vector.tensor_tensor(
                                out=dBu[:, j * T:(j + 1) * T], in0=dx[et],
                                in1=bcB[:, j * T:(j + 1) * T], op=OP.mult)
                        dA3 = dA.rearrange("p (g t) -> p g t", g=NG)
                        dBu3 = dBu.rearrange("p (g t) -> p g t", g=NG)
                        if c > 0:
                            tmpc = scp.tile([128, NG], BF16, tag="tmpc")
                            nc.vector.tensor_tensor(
                                out=tmpc, in0=dA3[:, :, 0],
                                in1=carry[et][:, n0:n0 + NG], op=OP.mult)
                            nc.vector.tensor_tensor(
                                out=dBu3[:, :, 0], in0=tmpc,
                                in1=dBu3[:, :, 0], op=OP.add)
                        nc.vector.memset(dA3[:, :, 0], 0.0)
                        nc.vector.tensor_tensor_scan(
                            out=h, data0=dA, data1=dBu, initial=0.0,
                            op0=OP.mult, op1=OP.add)
                        h3 = h.rearrange("p (g t) -> p g t", g=NG)
                        nc.vector.tensor_copy(
                            out=carry[et][:, n0:n0 + NG],
                            in_=h3[:, :, T - 1])
                        nc.vector.tensor_tensor(
                            out=g, in0=h, in1=bcC, op=OP.mult)
                        for j in range(NG):
                            nc.tensor.matmul(
                                y_ps[et], id_sb, g[:, j * T:(j + 1) * T],
                                start=(grp == 0 and j == 0),
                                stop=(grp == NGRP - 1 and j == NG - 1))
                # skip + gate: y = (y + x*D) * silu(z)
                y = []
                for et in range(NB_E2):
                    ysb = sp.tile([128, T], BF16, tag=f"ysb{et}")
                    nc.scalar.copy(out=ysb, in_=y_ps[et])
                    nc.vector.scalar_tensor_tensor(
                        out=ysb, in0=xT[et], scalar=Dp_sb[et][:, 0:1],
                        in1=ysb, op0=OP.mult, op1=OP.add)
                    nc.vector.tensor_tensor(
                        out=ysb, in0=ysb, in1=sz[et], op=OP.mult)
                    y.append(ysb)
                # out_proj -> out[:, chunk]
                for m in range(DM // 128):
                    acco = ps_main.tile([128, T], F32, tag="mainps")
                    for k in range(NB_E2):
                        nc.tensor.matmul(
                            acco, w_out_sb[k][:, m * 128:(m + 1) * 128],
                            y[k], start=(k == 0), stop=(k == 3))
                    osb = sp.tile([128, T], BF16, tag="osb")
                    nc.scalar.copy(out=osb, in_=acco)
                    nc.sync.dma_start(
                        out[m * 128:(m + 1) * 128, t0:t0 + T], osb)
    _split_dma_waits(nc)
    return nc


@functools.lru_cache(maxsize=1)
def _get_program():
    return build_core_program()


def _prep_core_inputs(hs, in_w, out_w, conv_w, conv_b, xproj_w, dt_w, dt_b,
                      A_log, D, b, rev, eh):
    bf = ml_dtypes.bfloat16
    own = slice(eh * E2, (eh + 1) * E2)
    perm = np.r_[np.arange(eh * E2, (eh + 1) * E2),
                 np.arange((1 - eh) * E2, (2 - eh) * E2)]
    u = hs[b] if not rev else hs[b, ::-1]
    u_t = np.ascontiguousarray(u.T).astype(bf)                  # [DM, L]
    w_x = in_w[:EF][perm]                                       # [EF, DM]
    w_z = in_w[EF:][own]                                        # [E2, DM]
    w_in_t = np.ascontiguousarray(
        np.concatenate([w_x, w_z], 0).T).astype(bf)             # [DM, 1536]
    cw = conv_w[:, 0, :][perm]                                  # [EF, 4]
    dg = np.zeros((4, NB_EF, 128, 128), bf)
    for k in range(4):
        for eb in range(NB_EF):
            dg[k, eb] = np.diag(cw[eb * 128:(eb + 1) * 128, k]).astype(bf)
    cb = np.ascontiguousarray(conv_b[perm][:, None]).astype(np.float32)
    xp = xproj_w[:, perm]                                       # [64, EF]
    xp_pad = np.zeros((96, EF), np.float32)
    xp_pad[0:32] = xp[0:32]        # dt
    xp_pad[32:48] = xp[32:48]      # B
    xp_pad[64:80] = xp[48:64]      # C
    w_xp_t = np.ascontiguousarray(xp_pad.T).astype(bf)          # [EF, 96]
    w_dt_t = np.ascontiguousarray(dt_w[own].T).astype(bf)       # [R, E2]
    dtb = np.ascontiguousarray(dt_b[own][:, None]).astype(np.float32)
    A = (-np.exp(A_log[own])).astype(np.float32)                # [E2, NST]
    Dpv = np.ascontiguousarray(D[own][:, None]).astype(np.float32)
    w_out_t = np.ascontiguousarray(out_w[:, own].T).astype(bf)
    ohm = np.zeros((96, NST * 128), np.float32)
    for n in range(NST):
        ohm[32 + n, n * 128:(n + 1) * 128] = 1.0
        ohm[64 + n, n * 128:(n + 1) * 128] = 1.0
    idm = np.eye(128, dtype=np.float32)
    return dict(u=u_t, w_in=w_in_t, dg=dg, conv_b=cb, w_xp=w_xp_t,
                w_dt=w_dt_t, dt_b=dtb, A=A, Dp=Dpv, w_out=w_out_t,
                oh=ohm.astype(bf), idm=idm.astype(bf))


def kernel(hidden_states, in_proj_w, out_proj_w,
           conv_w_f, conv_b_f, xproj_w_f, dtproj_w_f, dtproj_b_f, A_log_f,
           D_f, conv_w_r, conv_b_r, xproj_w_r, dtproj_w_r, dtproj_b_r,
           A_log_r, D_r, _results_hook=None):
    hs = np.asarray(hidden_states, np.float32)
    params = {
        False: (conv_w_f, conv_b_f, xproj_w_f, dtproj_w_f, dtproj_b_f,
                A_log_f, D_f),
        True: (conv_w_r, conv_b_r, xproj_w_r, dtproj_w_r, dtproj_b_r,
               A_log_r, D_r),
    }
    cores = []          # (b, rev, eh)
    in_maps = []
    for b in range(2):
        for rev in (False, True):
            for eh in range(2):
                cw, cb, xw, dw, db, al, dd = [np.asarray(p, np.float32)
                                              for p in params[rev]]
                in_maps.append(_prep_core_inputs(
                    hs, np.asarray(in_proj_w, np.float32),
                    np.asarray(out_proj_w, np.float32),
                    cw, cb, xw, dw, db, al, dd, b, rev, eh))
                cores.append((b, rev, eh))
    nc = _get_program()
    res = run_bass_kernel_spmd(nc, in_maps, core_ids=list(range(8)),
                               trace=TRACE)
    if _results_hook is not None:
        _results_hook(res)
    out = np.zeros((2, L, DM), np.float32)
    for (b, rev, eh), r in zip(cores, res.results):
        part = np.asarray(r["out"], np.float32).T      # [L, DM]
        if rev:
            part = part[::-1]
        out[b] += part
    return out


# revision 8
# speedup vs baseline: 1.2422x; 1.1222x over previous
"""BiMamba (bidirectional Mamba-1) Trainium2 kernel.

Full inputs -> full outputs. Sharding: 8 cores = (batch 2) x (direction 2) x
(channel-half 2); every core runs the SAME NEFF on different inputs (the
host reverses the sequence for the backward direction, permutes channel
halves, and pre-transposes weights/activations to channel-major).

Per-core pipeline (channel-major [channel, time], chunked over time).
Engine assignment tuned from HW traces (DVE is the bottleneck):
  PE:   in_proj, depthwise conv as diagonal matmuls, x_proj, dt_proj,
        B/C one-hot partition broadcasts, y state-sum via identity-matmul
        PSUM accumulation, out_proj  (all bf16)
  ACT:  softplus = Ln(1+Exp(.)), SiLU, dA = Exp(A*delta), PSUM->SBUF
        bf16 copies
  DVE:  dBu = dx*B and g = h*C as bf16 2x-mode TTs over 4-state
        concatenated tiles, the hardware linear scan (fp32 internal
        state) over [128, 4*T] with per-state dA-column resets, carry
        fold/extract as tiny strided ops
"""

import functools

import ml_dtypes
import numpy as np

import concourse.bass as bass
import concourse.mybir as mybir
import concourse.tile as tile
from concourse.tile import add_dep_helper
from concourse.bass_utils import run_bass_kernel_spmd

L = 8192          # sequence length
DM = 512          # d_model
EF = 1024         # d_inner (full)
E2 = 512          # d_inner per core (channel-half)
NST = 16          # d_state
R = 32            # dt_rank
T = 512           # time chunk
NCH = L // T
NG = 4            # states per scan group
NGRP = NST // NG  # groups
NB_E2 = E2 // 128     # 4 e-tiles for own half
NB_EF = EF // 128     # 8 e-tiles full
F32 = mybir.dt.float32
BF16 = mybir.dt.bfloat16
AF = mybir.ActivationFunctionType
OP = mybir.AluOpType
TRACE = False      # set by test harness to capture an NTFF profile


def _split_dma_waits(nc):
    """walrus codegen embeds a limited number of sem-waits per instruction
    (1 for dynamic DMA descriptors, 2 for compute engine instructions);
    move extra waits onto NoOps executed by the same engine just before."""
    SKIP = {"EventSemaphore", "UnconditionalBranch", "Call",
            "RegisterMove", "NoOp", "ISA"}
    f0 = nc.m.functions[0]
    plan = {}
    for blk in f0.blocks:
        for inst in list(blk.instructions):
            if inst.opcode in SKIP or inst.sync_info is None:
                continue
            cap = 1
            if len(inst.sync_info.on_wait) > cap:
                plan[inst.name] = (inst, cap)
    if not plan:
        return
    existing = {i.name for blk in f0.blocks for i in blk.instructions}

    def new_nop(engine, waits):
        nc.engines[engine].nop()
        newn = None
        for blk in f0.blocks:
            for i in blk.instructions:
                if i.name not in existing:
                    newn = i
                    existing.add(i.name)
        assert newn is not None
        newn.sync_info = mybir.SyncInfo(on_wait=waits, on_update=[])
        return newn

    nops_for = {}
    for name, (inst, cap) in plan.items():
        si = inst.sync_info
        waits = list(si.on_wait)
        if inst.opcode == "DMACopy":
            keep = [w for w in waits if "DMA" in (w.ant_name or "")][-cap:]
            if not keep:
                keep = waits[-cap:]
        else:
            keep = waits[-cap:]
        moved = [w for w in waits if not any(w is k for k in keep)]
        nops = []
        for i in range(0, len(moved), 1):
            nops.append(new_nop(inst.engine, moved[i:i + 1]))
        inst.sync_info = mybir.SyncInfo(on_wait=keep,
                                        on_update=list(si.on_update))
        nops_for[name] = nops
    nop_names = {n.name for ns in nops_for.values() for n in ns}
    for blk in f0.blocks:
        lst = []
        for inst in blk.instructions:
            if inst.name in nop_names:
                continue
            if inst.name in plan:
                lst.extend(nops_for[inst.name])
            lst.append(inst)
        blk.instructions = lst


def build_core_program(L=L, T=T, num_devices=8):
    NCH = L // T
    nc = bass.Bass("TRN2", target_bir_lowering=False, debug=False,
                   num_devices=num_devices)
    # ---- DRAM I/O (per core) ----
    u = nc.dram_tensor("u", [DM, L], BF16, kind="ExternalInput").ap()
    w_in = nc.dram_tensor("w_in", [DM, EF + E2], BF16,
                          kind="ExternalInput").ap()
    dg = nc.dram_tensor("dg", [4, NB_EF, 128, 128], BF16,
                        kind="ExternalInput").ap()
    conv_b = nc.dram_tensor("conv_b", [EF, 1], F32, kind="ExternalInput").ap()
    w_xp = nc.dram_tensor("w_xp", [EF, 96], BF16, kind="ExternalInput").ap()
    w_dt = nc.dram_tensor("w_dt", [R, E2], BF16, kind="ExternalInput").ap()
    dt_b = nc.dram_tensor("dt_b", [E2, 1], F32, kind="ExternalInput").ap()
    A_in = nc.dram_tensor("A", [E2, NST], F32, kind="ExternalInput").ap()
    Dp = nc.dram_tensor("Dp", [E2, 1], F32, kind="ExternalInput").ap()
    w_out = nc.dram_tensor("w_out", [E2, DM], BF16, kind="ExternalInput").ap()
    oh = nc.dram_tensor("oh", [96, NST * 128], BF16,
                        kind="ExternalInput").ap()
    idm = nc.dram_tensor("idm", [128, 128], BF16, kind="ExternalInput").ap()
    out = nc.dram_tensor("out", [DM, L], BF16, kind="ExternalOutput").ap()

    with tile.TileContext(nc) as tc:
        with (
            tc.tile_pool(name="wpool", bufs=1) as wp,
            tc.tile_pool(name="persist", bufs=1) as pp,
            tc.tile_pool(name="stream", bufs=2) as sp,
            tc.tile_pool(name="xother", bufs=2) as xo,
            tc.tile_pool(name="scanp", bufs=3) as scp,
            tc.tile_pool(name="bcpool", bufs=4) as bcp,
            tc.tile_pool(name="ps_main", bufs=2, space="PSUM") as ps_main,
            tc.tile_pool(name="ps_bc", bufs=2, space="PSUM") as ps_bc,
            tc.tile_pool(name="ps_y", bufs=2, space="PSUM") as ps_y,
        ):
            # ---------------- weights into SBUF ----------------
            w_in_sb = [wp.tile([128, EF + E2], BF16, tag=f"w_in{k}",
                               name=f"w_in{k}") for k in range(4)]
            for k in range(4):
                nc.sync.dma_start(w_in_sb[k], w_in[k * 128:(k + 1) * 128, :])
            dg_sb = [[wp.tile([128, 128], BF16, tag=f"dg{k}_{eb}",
                              name=f"dg{k}_{eb}")
                      for eb in range(NB_EF)] for k in range(4)]
            for k in range(4):
                for eb in range(NB_EF):
                    nc.sync.dma_start(dg_sb[k][eb], dg[k, eb])
            conv_b_sb = [wp.tile([128, 1], F32, tag=f"cb{eb}", name=f"cb{eb}")
                         for eb in range(NB_EF)]
            for eb in range(NB_EF):
                nc.sync.dma_start(conv_b_sb[eb],
                                  conv_b[eb * 128:(eb + 1) * 128, :])
            w_xp_sb = [wp.tile([128, 96], BF16, tag=f"wxp{eb}", name=f"wxp{eb}")
                       for eb in range(NB_EF)]
            for eb in range(NB_EF):
                nc.sync.dma_start(w_xp_sb[eb], w_xp[eb * 128:(eb + 1) * 128])
            w_dt_sb = wp.tile([R, E2], BF16)
            nc.sync.dma_start(w_dt_sb, w_dt)
            dt_b_sb = [wp.tile([128, 1], F32, tag=f"dtb{et}", name=f"dtb{et}")
                       for et in range(NB_E2)]
            A_sb = [wp.tile([128, NST], F32, tag=f"A{et}", name=f"A{et}")
                    for et in range(NB_E2)]
            Dp_sb = [wp.tile([128, 1], F32, tag=f"Dp{et}", name=f"Dp{et}")
                     for et in range(NB_E2)]
            for et in range(NB_E2):
                nc.sync.dma_start(dt_b_sb[et], dt_b[et * 128:(et + 1) * 128])
                nc.sync.dma_start(A_sb[et], A_in[et * 128:(et + 1) * 128])
                nc.sync.dma_start(Dp_sb[et], Dp[et * 128:(et + 1) * 128])
            w_out_sb = [wp.tile([128, DM], BF16, tag=f"wo{k}", name=f"wo{k}")
                        for k in range(NB_E2)]
            for k in range(NB_E2):
                nc.sync.dma_start(w_out_sb[k], w_out[k * 128:(k + 1) * 128])
            oh_sb = wp.tile([96, NST * 128], BF16)
            nc.sync.dma_start(oh_sb, oh)
            id_sb = wp.tile([128, 128], BF16)
            nc.sync.dma_start(id_sb, idm)

            # persistent state
            xpre_sb = [pp.tile([128, T + 3], BF16, tag=f"xpre{eb}",
                               name=f"xpre{eb}") for eb in range(NB_EF)]
            carry = [pp.tile([128, NST], BF16, tag=f"carry{et}",
                             name=f"carry{et}") for et in range(NB_E2)]
            for eb in range(NB_EF):
                nc.vector.memset(xpre_sb[eb][:, 0:3], 0.0)

            last_exp_of_chunk = None
            # ---------------- chunk loop ----------------
            for c in range(NCH):
                t0 = c * T
                # halo: carry last 3 cols of previous chunk
                if c > 0:
                    for eb in range(NB_EF):
                        nc.vector.tensor_copy(
                            out=xpre_sb[eb][:, 0:3],
                            in_=xpre_sb[eb][:, T:T + 3])
                # u chunk in
                u_sb = [sp.tile([128, T], BF16, tag=f"u{k}", name=f"u{k}")
                        for k in range(4)]
                for k in range(4):
                    nc.sync.dma_start(
                        u_sb[k], u[k * 128:(k + 1) * 128, t0:t0 + T])
                # in_proj: 12 m-blocks (8 x-part, 4 z-own)
                sz = []
                silu_insts = []
                for m in range(12):
                    acc = ps_main.tile([128, T], F32, tag="mainps")
                    for k in range(4):
                        nc.tensor.matmul(
                            acc, w_in_sb[k][:, m * 128:(m + 1) * 128],
                            u_sb[k], start=(k == 0), stop=(k == 3))
                    if m < 8:
                        nc.scalar.copy(out=xpre_sb[m][:, 3:T + 3], in_=acc)
                    else:
                        szt = sp.tile([128, T], BF16, tag=f"sz{m - 8}")
                        si = nc.scalar.activation(out=szt, in_=acc,
                                                  func=AF.Silu)
                        silu_insts.append(si)
                        sz.append(szt)
                # conv (diag matmuls) + bias + silu -> xT
                xT = []
                for eb in range(NB_EF):
                    accc = ps_main.tile([128, T], F32, tag="mainps")
                    for k in range(4):
                        nc.tensor.matmul(
                            accc, dg_sb[k][eb], xpre_sb[eb][:, k:k + T],
                            start=(k == 0), stop=(k == 3))
                    pool = sp if eb < NB_E2 else xo
                    xt = pool.tile([128, T], BF16, tag=f"xT{eb}")
                    si = nc.scalar.activation(out=xt, in_=accc, func=AF.Silu,
                                              bias=conv_b_sb[eb][:, 0:1])
                    silu_insts.append(si)
                    xT.append(xt)
                # keep ACT table sets clustered: all Silus of this chunk
                # after the previous chunk's last Exp
                if last_exp_of_chunk is not None:
                    for si in silu_insts:
                        add_dep_helper(si.ins, last_exp_of_chunk.ins,
                                       sync=False,
                                       reason="act table-set clustering")
                # x_proj -> xdbl [96, T] -> bf16
                xdbl_ps = ps_main.tile([96, T], F32, tag="mainps")
                for eb in range(NB_EF):
                    nc.tensor.matmul(xdbl_ps, w_xp_sb[eb], xT[eb],
                                     start=(eb == 0), stop=(eb == 7))
                xdbl = sp.tile([96, T], BF16, tag="xdbl")
                nc.scalar.copy(out=xdbl, in_=xdbl_ps)
                # dt_proj + softplus -> delta; dx = delta*x (per e-tile)
                delta = []
                dx = []
                for et in range(NB_E2):
                    dpre = ps_main.tile([128, T], F32, tag="mainps")
                    nc.tensor.matmul(
                        dpre, w_dt_sb[:, et * 128:(et + 1) * 128],
                        xdbl[0:R, :], start=True, stop=True)
                    dl = sp.tile([128, T], BF16, tag=f"delta{et}")
                    # e = exp(pre + dt_b); delta = ln(1 + e)
                    nc.scalar.activation(out=dl, in_=dpre, func=AF.Exp,
                                         bias=dt_b_sb[et][:, 0:1])
                    nc.scalar.activation(out=dl, in_=dl, func=AF.Ln,
                                         bias=1.0)
                    delta.append(dl)
                    dxt = sp.tile([128, T], BF16, tag=f"dx{et}")
                    nc.vector.tensor_tensor(out=dxt, in0=dl, in1=xT[et],
                                            op=OP.mult)
                    dx.append(dxt)
                # B/C broadcasts for all 16 states, formed once per chunk:
                # one-hot matmuls land state-pairs in [128, 2*T] PSUM tiles,
                # one ACT copy moves each pair into the bf16 group tiles
                bcB = [bcp.tile([128, NG * T], BF16, tag="bcB",
                                name=f"bcB{grp}") for grp in range(NGRP)]
                bcC = [bcp.tile([128, NG * T], BF16, tag="bcC",
                                name=f"bcC{grp}") for grp in range(NGRP)]
                for grp in range(NGRP):
                    for half in range(NG // 2):
                        pb = ps_bc.tile([128, 2 * T], F32, tag="bcps")
                        pc2 = ps_bc.tile([128, 2 * T], F32, tag="bcps")
                        for j2 in range(2):
                            n = grp * NG + half * 2 + j2
                            nc.tensor.matmul(
                                pb[:, j2 * T:(j2 + 1) * T],
                                oh_sb[32:48, n * 128:(n + 1) * 128],
                                xdbl[32:48, :], start=True, stop=True)
                            nc.tensor.matmul(
                                pc2[:, j2 * T:(j2 + 1) * T],
                                oh_sb[64:80, n * 128:(n + 1) * 128],
                                xdbl[64:80, :], start=True, stop=True)
                        o0 = half * 2 * T
                        nc.scalar.copy(
                            out=bcB[grp][:, o0:o0 + 2 * T], in_=pb)
                        nc.scalar.copy(
                            out=bcC[grp][:, o0:o0 + 2 * T], in_=pc2)
                # scan: et-outer, 4-state groups inner
                y = []
                for et in range(NB_E2):
                    y_ps = ps_y.tile([128, T], F32, tag="yps",
                                     name="yps")
                    for grp in range(NGRP):
                        n0 = grp * NG
                        dA = scp.tile([128, NG * T], BF16, tag="dA")
                        dBu = scp.tile([128, NG * T], BF16, tag="dBu")
                        h = scp.tile([128, NG * T], BF16, tag="h")
                        g = scp.tile([128, NG * T], BF16, tag="g")
                        for j in range(NG):
                            n = n0 + j
                            ei = nc.scalar.activation(
                                out=dA[:, j * T:(j + 1) * T],
                                in_=delta[et], func=AF.Exp,
                                scale=A_sb[et][:, n:n + 1])
                            last_exp_of_chunk = ei
                        dA3 = dA.rearrange("p (g t) -> p g t", g=NG)
                        dBu3 = dBu.rearrange("p (g t) -> p g t", g=NG)
                        bcB3 = bcB[grp].rearrange("p (g t) -> p g t", g=NG)
                        nc.vector.tensor_tensor(
                            out=dBu3,
                            in0=dx[et].unsqueeze(1).broadcast_to(
                                [128, NG, T]),
                            in1=bcB3, op=OP.mult)
                        if c > 0:
                            tmpc = scp.tile([128, NG], BF16, tag="tmpc")
                            nc.vector.tensor_tensor(
                                out=tmpc, in0=dA3[:, :, 0],
                                in1=carry[et][:, n0:n0 + NG], op=OP.mult)
                            nc.vector.tensor_tensor(
                                out=dBu3[:, :, 0], in0=tmpc,
                                in1=dBu3[:, :, 0], op=OP.add)
                        nc.vector.memset(dA3[:, :, 0], 0.0)
                        nc.vector.tensor_tensor_scan(
                            out=h, data0=dA, data1=dBu, initial=0.0,
                            op0=OP.mult, op1=OP.add)
                        h3 = h.rearrange("p (g t) -> p g t", g=NG)
                        nc.vector.tensor_copy(
                            out=carry[et][:, n0:n0 + NG],
                            in_=h3[:, :, T - 1])
                        nc.vector.tensor_tensor(
                            out=g, in0=h, in1=bcC[grp], op=OP.mult)
                        for j in range(NG):
                            nc.tensor.matmul(
                                y_ps, id_sb, g[:, j * T:(j + 1) * T],
                                start=(grp == 0 and j == 0),
                                stop=(grp == NGRP - 1 and j == NG - 1))
                    # skip + gate: y = (y + x*D) * silu(z)
                    ysb = sp.tile([128, T], BF16, tag=f"ysb{et}",
                                  name=f"ysb{et}")
                    nc.vector.scalar_tensor_tensor(
                        out=ysb, in0=xT[et], scalar=Dp_sb[et][:, 0:1],
                        in1=y_ps, op0=OP.mult, op1=OP.add)
                    nc.vector.tensor_tensor(
                        out=ysb, in0=ysb, in1=sz[et], op=OP.mult)
                    y.append(ysb)
                # out_proj -> out[:, chunk]
                for m in range(DM // 128):
                    acco = ps_main.tile([128, T], F32, tag="mainps")
                    for k in range(NB_E2):
                        nc.tensor.matmul(
                            acco, w_out_sb[k][:, m * 128:(m + 1) * 128],
                            y[k], start=(k == 0), stop=(k == 3))
                    osb = sp.tile([128, T], BF16, tag="osb")
                    nc.scalar.copy(out=osb, in_=acco)
                    nc.sync.dma_start(
                        out[m * 128:(m + 1) * 128, t0:t0 + T], osb)
    _split_dma_waits(nc)
    return nc


@functools.lru_cache(maxsize=1)
def _get_program():
    return build_core_program()


def _prep_core_inputs(hs, in_w, out_w, conv_w, conv_b, xproj_w, dt_w, dt_b,
                      A_log, D, b, rev, eh):
    bf = ml_dtypes.bfloat16
    own = slice(eh * E2, (eh + 1) * E2)
    perm = np.r_[np.arange(eh * E2, (eh + 1) * E2),
                 np.arange((1 - eh) * E2, (2 - eh) * E2)]
    u = hs[b] if not rev else hs[b, ::-1]
    u_t = np.ascontiguousarray(u.T).astype(bf)                  # [DM, L]
    w_x = in_w[:EF][perm]                                       # [EF, DM]
    w_z = in_w[EF:][own]                                        # [E2, DM]
    w_in_t = np.ascontiguousarray(
        np.concatenate([w_x, w_z], 0).T).astype(bf)             # [DM, 1536]
    cw = conv_w[:, 0, :][perm]                                  # [EF, 4]
    dg = np.zeros((4, NB_EF, 128, 128), bf)
    for k in range(4):
        for eb in range(NB_EF):
            dg[k, eb] = np.diag(cw[eb * 128:(eb + 1) * 128, k]).astype(bf)
    cb = np.ascontiguousarray(conv_b[perm][:, None]).astype(np.float32)
    xp = xproj_w[:, perm]                                       # [64, EF]
    xp_pad = np.zeros((96, EF), np.float32)
    xp_pad[0:32] = xp[0:32]        # dt
    xp_pad[32:48] = xp[32:48]      # B
    xp_pad[64:80] = xp[48:64]      # C
    w_xp_t = np.ascontiguousarray(xp_pad.T).astype(bf)          # [EF, 96]
    w_dt_t = np.ascontiguousarray(dt_w[own].T).astype(bf)       # [R, E2]
    dtb = np.ascontiguousarray(dt_b[own][:, None]).astype(np.float32)
    A = (-np.exp(A_log[own])).astype(np.float32)                # [E2, NST]
    Dpv = np.ascontiguousarray(D[own][:, None]).astype(np.float32)
    w_out_t = np.ascontiguousarray(out_w[:, own].T).astype(bf)
    ohm = np.zeros((96, NST * 128), np.float32)
    for n in range(NST):
        ohm[32 + n, n * 128:(n + 1) * 128] = 1.0
        ohm[64 + n, n * 128:(n + 1) * 128] = 1.0
    idm = np.eye(128, dtype=np.float32)
    return dict(u=u_t, w_in=w_in_t, dg=dg, conv_b=cb, w_xp=w_xp_t,
                w_dt=w_dt_t, dt_b=dtb, A=A, Dp=Dpv, w_out=w_out_t,
                oh=ohm.astype(bf), idm=idm.astype(bf))


def kernel(hidden_states, in_proj_w, out_proj_w,
           conv_w_f, conv_b_f, xproj_w_f, dtproj_w_f, dtproj_b_f, A_log_f,
           D_f, conv_w_r, conv_b_r, xproj_w_r, dtproj_w_r, dtproj_b_r,
           A_log_r, D_r, _results_hook=None):
    hs = np.asarray(hidden_states, np.float32)
    params = {
        False: (conv_w_f, conv_b_f, xproj_w_f, dtproj_w_f, dtproj_b_f,
                A_log_f, D_f),
        True: (conv_w_r, conv_b_r, xproj_w_r, dtproj_w_r, dtproj_b_r,
               A_log_r, D_r),
    }
    cores = []          # (b, rev, eh)
    in_maps = []
    for b in range(2):
        for rev in (False, True):
            for eh in range(2):
                cw, cb, xw, dw, db, al, dd = [np.asarray(p, np.float32)
                                              for p in params[rev]]
                in_maps.append(_prep_core_inputs(
                    hs, np.asarray(in_proj_w, np.float32),
                    np.asarray(out_proj_w, np.float32),
                    cw, cb, xw, dw, db, al, dd, b, rev, eh))
                cores.append((b, rev, eh))
    nc = _get_program()
    res = run_bass_kernel_spmd(nc, in_maps, core_ids=list(range(8)),
                               trace=TRACE)
    if _results_hook is not None:
        _results_hook(res)
    out = np.zeros((2, L, DM), np.float32)
    for (b, rev, eh), r in zip(cores, res.results):
        part = np.asarray(r["out"], np.float32).T      # [L, DM]
        if rev:
            part = part[::-1]
        out[b] += part
    return out
